# revision 32
# baseline (speedup 1.0000x reference)
"""AGRNN GNN message-passing kernel for 8 TRN2 NeuronCores (Bass/Tile).

Strategy (dst-sharded edge partition, v4 -- 1.93ms HW, rel err 5.8e-3):
  - Sort edges by dst on host; core r owns nodes [r*2500,(r+1)*2500) and all
    edges pointing into them, grouped into 128-node windows with edge tiles
    of 128 (padded).  All per-destination segment ops become per-window
    one-hot matmuls on TensorE -- no scatter.
  - Phase 0: per-node SOURCE table comb[n] = [n_f@We_src | w2v@Wel_src]
    in PLAIN (unfolded) space, stored FP8E4 (halves AllGather + gather
    bytes; plain space keeps fp8 well-scaled), plus LOCAL bf16 dst tables
    dstC[n] = [n_f@We_dst + b_e | w2v@Wel_dst + b_el].  AllGather is
    chunked (CHW=5 windows/chunk, per-chunk DRAM tensors so Tile's
    per-tensor dep tracking lets chunks overlap); a tiny warm-up
    AllGather up front absorbs the cross-core startup barrier.
  - w2v[src] is host-pre-gathered (index prep) so the lang aggregation rhs
    streams from DRAM instead of riding the gather.
  - Phase 1+2: software-pipelined window loop (iteration w runs agg
    MMs(w-1) -> readout(w-2) -> norm(w-1) -> pass A(w) -> node MLP(w-1))
    to keep the PE queue dense.  Pass A per 4-tile group: one batched
    dma_gather of comb[src] (SWDGE per-row cost ~9ns dominates, so batch);
    per tile one-hot expand dstC + s_f term + gathered rows (identity MM)
    into PSUM; ACT Relu -> efc / scrB; DVE STT+accum against broadcast
    W_a / W_al rows gives both attention logits (signed, plain space).
    Window epilogue: leaky-relu + one Exp over [128, 2K] logits, then efc
    and wv rows are scaled IN PLACE so pass B is pure PE aggregation.
  - Phase 3 per window, transposed frame (no DMA transposes): per 4-tile
    group, dma_gather(transpose=True) delivers srcR[src]^T directly in
    [128, 4 chunks, edges] layout (bf16: fp8 transpose-gathers interleave
    byte pairs, and >512 idxs/op fails); prT chunks accumulate
    dstR^T-expansion (lhsT=dstRw chunk, rhs=one-hot MT) + s_f term
    (lhsT=Wr1_sf chunk, rhs=s_f^T) + gathered srcR^T (identity matmul);
    ACT Relu -> hT; final matmul lhsT=W_r2 chunk, rhs=hT chunk -> [13, e]
    PSUM; bias via vector -> packed [13, T*128] output columns; host
    unpermutes back to edge order.
"""
import sys

sys.path.insert(0, '/opt/trn_rl_repo')

import numpy as np

import concourse.bass as bass
import concourse.bacc as bacc
import concourse.tile as tile
from concourse import mybir
from concourse.masks import make_identity
from concourse.bass_utils import run_bass_kernel_spmd

P = 128
N, E = 20000, 320000
D, DW, DS, NCLS = 512, 300, 16, 13
DWP = 384            # DW padded to 3*128
WV = 302             # host-gathered w2v_src row: [w2v 300 | 1 | pad]
NCORES = 8
NPC = N // NCORES    # nodes per core
NW = (NPC + P - 1) // P          # windows per core (20; last = 68 nodes)
WSZ = [min(P, NPC - w * P) for w in range(NW)]
CHW = 5                          # windows per AllGather chunk
NCH = (NW + CHW - 1) // CHW      # collective chunks (4)
GRP = 4                          # edge tiles per batched gather (phase 1)
GRP3 = 4                         # edge tiles per batched gather (phase 3)
                                 # (transpose-gather fails at 8*128 idxs/op)
F32 = mybir.dt.float32
B16 = mybir.dt.bfloat16
I16 = mybir.dt.int16
F8 = mybir.dt.float8e4
AF = mybir.ActivationFunctionType
OP = mybir.AluOpType
DEBUG = False    # extra DRAM outputs for localization; off in production


# ----------------------------------------------------------------- host prep
def _prep(src, dst, s_f, word2vec=None):
    perm = np.argsort(dst, kind='stable')
    src_s = src[perm]
    dst_s = dst[perm]
    s_f_s = s_f[perm]

    lows = np.array([r * NPC + w * P for r in range(NCORES) for w in range(NW)]
                    + [N], dtype=np.int64)
    bnds = np.searchsorted(dst_s, lows)
    cnt = (bnds[1:] - bnds[:-1]).reshape(NCORES, NW)
    kw = np.maximum(1, (cnt + P - 1) // P).max(axis=0)    # [NW]
    T = int(kw.sum())
    tb = np.concatenate([[0], np.cumsum(kw)]).astype(np.int64)
    g8 = (np.concatenate([[0], np.cumsum(kw)]) * 8).astype(np.int64)  # col offs
    GW = int(g8[-1])                                      # gidx columns

    per_core = []
    for r in range(NCORES):
        srcm_slots = np.zeros((T, P), np.int32)   # chunk-major remapped
        dstl_slots = np.full((T, P), -1.0, np.float32)
        sf_slots = np.zeros((T, P, DS), np.float32)
        eid_slots = np.full((T, P), -1, np.int64)
        src_slots = np.zeros((T, P), np.int32)
        for w in range(NW):
            e0, e1 = bnds[r * NW + w], bnds[r * NW + w + 1]
            c = e1 - e0
            if c == 0:
                continue
            flat = tb[w] * P + np.arange(c)
            ti, pi = flat // P, flat % P
            sv = src_s[e0:e1]
            src_slots[ti, pi] = sv
            # chunk-major AllGather layout: chunk c of CHW windows, then rank
            sr, sl = sv // NPC, sv % NPC
            sw, sp = sl // P, sl % P
            srcm_slots[ti, pi] = ((sw // CHW) * (NCORES * CHW * P)
                                  + sr * (CHW * P) + (sw % CHW) * P + sp)
            dstl_slots[ti, pi] = (dst_s[e0:e1] - (r * NPC + w * P)).astype(np.float32)
            sf_slots[ti, pi] = s_f_s[e0:e1]
            eid_slots[ti, pi] = perm[e0:e1]
        # gather-index table (packed, exact-size gathers read no pads):
        # flat idx i of window w at [i%16, g8[w] + i//16]
        gidx = np.zeros((16, GW), np.int16)
        for w in range(NW):
            kr = int(kw[w])
            vals = srcm_slots[tb[w]:tb[w] + kr]
            blk = vals.reshape(-1).reshape(-1, 16).T.astype(np.int16)
            gidx[:, g8[w]:g8[w] + blk.shape[1]] = blk
        gidx_full = np.tile(gidx, (8, 1))          # replicate to 128 partitions
        pc = {
            'dstl_cols': np.ascontiguousarray(dstl_slots.T),
            'dstl_rows': np.ascontiguousarray(dstl_slots.reshape(1, T * P)),
            'sfT_cols': np.ascontiguousarray(
                sf_slots.transpose(2, 0, 1).reshape(DS, T * P)),
            'eid': eid_slots,
            'gidx': np.ascontiguousarray(gidx_full),
        }
        if word2vec is not None:
            wv = np.zeros((T, P, WV), np.float32)
            wv[:, :, :DW] = word2vec[src_slots]
            wv[:, :, DW] = 1.0
            pc['wv_cols'] = np.ascontiguousarray(
                wv.transpose(1, 0, 2).reshape(P, T * WV))
        per_core.append(pc)
    return per_core, [int(k) for k in kw], T


def _lang_split(W_al):
    """kept for test.py compat; returns pos-count of W_al."""
    w = np.asarray(W_al, np.float32).reshape(-1)
    pos = np.where(w >= 0)[0]
    neg = np.where(w < 0)[0]
    permc = np.concatenate([pos, neg])
    scale = np.abs(w)[permc]
    return permc, scale, int(len(pos))


# ------------------------------------------------------------- device build
def _build(T, kw, zero_ab=True, dp=D):
    nc = bacc.Bacc(None, target_bir_lowering=False, num_devices=NCORES)

    def inp(name, shape, dt=F32):
        return nc.declare_dram_parameter(name, list(shape), dt, isOutput=False)

    g8 = [0]
    for k in kw:
        g8.append(g8[-1] + k * 8)
    GW = g8[-1]

    nfT_loc = inp('nfT_loc', (D, NPC), B16)
    w2vT_loc = inp('w2vT_loc', (DWP, NPC), B16)
    We_src = inp('We_src', (D, D), B16)       # |W_a|-folded, sign-permuted
    We_sf = inp('We_sf', (DS, D), B16)        # folded/permuted
    We_dst = inp('We_dst', (D, D), B16)       # folded/permuted
    Wel_src = inp('Wel_src', (DWP, D), B16)   # plain
    Wel_dst = inp('Wel_dst', (DWP, D), B16)   # plain
    Wn_nf = inp('Wn_nf', (D, D), B16)
    Wn_z = inp('Wn_z', (D, D), B16)           # unscale+perm folded into rows
    Wnl_w = inp('Wnl_w', (DWP, DW), B16)
    Wnl_z = inp('Wnl_z', (DWP, DW), B16)
    Wr1_nsrc = inp('Wr1_nsrc', (D, D), B16)
    Wr1_wsrc = inp('Wr1_wsrc', (DWP, D), B16)
    Wr1_sf = inp('Wr1_sf', (DS, D), B16)
    Wr1_wdst = inp('Wr1_wdst', (DWP, D), B16)
    Wr1_ndst = inp('Wr1_ndst', (D, D), B16)
    W_r2 = inp('W_r2', (D, NCLS), B16)
    b_e = inp('b_e', (1, D), B16)             # folded/permuted
    b_el = inp('b_el', (1, D), B16)
    b_n = inp('b_n', (1, D), B16)
    b_nl = inp('b_nl', (1, DW), B16)
    b_r1 = inp('b_r1', (1, D), B16)
    b_r2col = inp('b_r2col', (NCLS, 1), F32)
    Wal_row = inp('Wal_row', (1, D), B16)     # signed, for lang logit STT
    Wa_row = inp('Wa_row', (1, D), B16)       # signed, for e_f logit STT
    gidx_in = inp('gidx', (P, GW), I16)
    dstl_cols = inp('dstl_cols', (P, T), B16)
    dstl_rows = inp('dstl_rows', (1, T * P), B16)
    sfT_cols = inp('sfT_cols', (DS, T * P), B16)
    wv_cols = inp('wv_cols', (P, T * WV), B16)
    out_colsT = nc.declare_dram_parameter('out_colsT', [NCLS, T * P], B16,
                                          isOutput=True)
    if DEBUG:
        dbg_comb = nc.declare_dram_parameter('dbg_comb', [4 * P, 2 * D], B16,
                                             isOutput=True)
        dbg_gth = nc.declare_dram_parameter('dbg_gth', [P, 4 * 2 * D], B16,
                                            isOutput=True)
        dbg_srcR = nc.declare_dram_parameter('dbg_srcR', [2 * P, D], B16,
                                             isOutput=True)
        dbg_dstR = nc.declare_dram_parameter('dbg_dstR', [P, D], B16,
                                             isOutput=True)
        dbg_hT = nc.declare_dram_parameter('dbg_hT', [P, 4 * 512], B16,
                                           isOutput=True)

    CRI = CHW * P                # rows contributed per core per chunk
    CRO = NCORES * CRI           # rows produced per chunk
    # per-chunk input tensors so each AllGather only depends on its own
    # chunk's writes (whole-tensor dep tracking would serialize them)
    combA_in = [nc.dram_tensor(f'combA_in{c}', [CRI, 2 * D], F8)
                for c in range(NCH)]
    combA = nc.dram_tensor('combA', [NCH * NCORES * CHW * P, 2 * D], F8,
                           addr_space='Shared')
    dstC_dram = nc.dram_tensor('dstC_dram', [NW * P, 2 * D], B16)
    srcR_in = [nc.dram_tensor(f'srcR_in{c}', [CRI, D], B16)
               for c in range(NCH)]
    srcR_full = nc.dram_tensor('srcR_full', [NCH * NCORES * CHW * P, D], B16,
                               addr_space='Shared')
    dstR_dram = nc.dram_tensor('dstR_dram', [NW * P, D], B16)
    warm_in = nc.dram_tensor('warm_in', [1, 64], B16)
    warm_out = nc.dram_tensor('warm_out', [NCORES, 64], B16,
                              addr_space='Shared')

    PA = dp                      # positive-sign column count of W_a

    with tile.TileContext(nc, num_cores=NCORES) as tc:
        with tc.tile_pool(name='const', bufs=1) as cp, \
             tc.tile_pool(name='wpool', bufs=1) as wp, \
             tc.tile_pool(name='win', bufs=2) as wn, \
             tc.tile_pool(name='wbig', bufs=2) as wb, \
             tc.tile_pool(name='wout', bufs=1) as wo, \
             tc.tile_pool(name='efcp', bufs=2) as efp, \
             tc.tile_pool(name='win1', bufs=2) as w1, \
             tc.tile_pool(name='gthp', bufs=2) as gp, \
             tc.tile_pool(name='edge', bufs=3) as ep, \
             tc.tile_pool(name='ph3', bufs=3) as p3, \
             tc.tile_pool(name='pagg', bufs=1, space='PSUM') as pagg, \
             tc.tile_pool(name='pbig', bufs=4, space='PSUM') as pbig, \
             tc.tile_pool(name='psml', bufs=2, space='PSUM') as psml:

            # ---------------- constants
            ident = cp.tile([P, P], B16)
            make_identity(nc, ident[:])
            ones_row = cp.tile([1, P], B16)
            nc.vector.memset(ones_row[:], 1.0)
            pcol = cp.tile([P, 1], F32)
            nc.gpsimd.iota(pcol[:], pattern=[[0, 1]], base=0,
                           channel_multiplier=1,
                           allow_small_or_imprecise_dtypes=True)
            KMAX = max(kw)
            colidxK = cp.tile([P, KMAX * P], B16)
            nc.gpsimd.iota(colidxK[:], pattern=[[0, KMAX], [1, P]], base=0,
                           channel_multiplier=0,
                           allow_small_or_imprecise_dtypes=True)
            Wal_b = cp.tile([P, D], B16)
            nc.sync.dma_start(Wal_b[:], Wal_row[:].to_broadcast((P, D)))
            Wa_b = cp.tile([P, D], B16)
            nc.sync.dma_start(Wa_b[:], Wa_row[:].to_broadcast((P, D)))
            br2c = cp.tile([NCLS, 1], F32)
            nc.sync.dma_start(br2c[:], b_r2col[:])
            gidx = cp.tile([P, GW], I16)
            nc.sync.dma_start(gidx[:], gidx_in[:])

            # tiny warm-up collective: absorbs the cross-core startup
            # barrier while phase-0 compute proceeds
            nc.gpsimd.collective_compute(
                'AllGather', OP.bypass,
                replica_groups=[list(range(NCORES))],
                ins=[warm_in[:]], outs=[warm_out[:]])

            # ---------------- weights ([128, nchunk, ncols] tiles)
            def wload(t, nrow, ncol, name):
                if nrow == DS:
                    tl = wp.tile([DS, ncol], B16, tag=name)
                    nc.sync.dma_start(tl[:], t[:])
                    return [tl[:]]
                nch = nrow // P
                tl = wp.tile([P, nch, ncol], B16, tag=name)
                nc.sync.dma_start(tl[:], t[:].rearrange('(c p) n -> p c n', p=P))
                return [tl[:, c] for c in range(nch)]

            # phase-0-only weights share slots with weights loaded after
            # phase 0 (same tags).
            wesrc = wload(We_src, D, D, 'wA')
            welsrc = wload(Wel_src, DWP, D, 'wB')
            wedst = wload(We_dst, D, D, 'wC')
            weldst = wload(Wel_dst, DWP, D, 'wD')
            wesf = wload(We_sf, DS, D, 'wesf')
            wnlw = wload(Wnl_w, DWP, DW, 'wnlw')
            wnlz = wload(Wnl_z, DWP, DW, 'wnlz')
            wr1ns = wload(Wr1_nsrc, D, D, 'wr1ns')
            wr1sf = wload(Wr1_sf, DS, D, 'wr1sf')
            wr1nd = wload(Wr1_ndst, D, D, 'wr1nd')
            wr2 = wload(W_r2, D, NCLS, 'wr2')
            bias_sb = {}
            for nm, t, wdt in (('b_e', b_e, D), ('b_el', b_el, D),
                               ('b_n', b_n, D), ('b_nl', b_nl, DW),
                               ('b_r1', b_r1, D)):
                bt = wp.tile([1, wdt], B16, tag='bias_' + nm)
                nc.sync.dma_start(bt[:], t[:])
                bias_sb[nm] = bt

            def bias_mm(ps_ap, nm, stop=True):
                nc.tensor.matmul(ps_ap, lhsT=ones_row[:],
                                 rhs=bias_sb[nm][:], start=False, stop=stop)

            # ============ PHASE 0: src table shard + local dst tables
            for g in range(NW):
                lo = g * P
                sz = WSZ[g]
                nft = wn.tile([P, 4, P], B16, tag='nft')
                w2t = wn.tile([P, 3, P], B16, tag='w2t')
                if sz < P:
                    nc.vector.memset(nft[:], 0.0)
                    nc.vector.memset(w2t[:], 0.0)
                nc.sync.dma_start(
                    nft[:, :, :sz],
                    nfT_loc[:, lo:lo + sz].rearrange('(c p) n -> p c n', p=P))
                nc.sync.dma_start(
                    w2t[:, :, :sz],
                    w2vT_loc[:, lo:lo + sz].rearrange('(c p) n -> p c n', p=P))
                ps_a = pbig.tile([P, D], F32, space='PSUM', tag='pbig')
                ps_b = pbig.tile([P, D], F32, space='PSUM', tag='pbig')
                for c in range(4):
                    nc.tensor.matmul(ps_a[:], lhsT=nft[:, c], rhs=wesrc[c],
                                     start=(c == 0), stop=(c == 3))
                for c in range(3):
                    nc.tensor.matmul(ps_b[:], lhsT=w2t[:, c], rhs=welsrc[c],
                                     start=(c == 0), stop=(c == 2))
                cs = wn.tile([P, 2 * D], F8, tag='cs8')
                nc.scalar.copy(cs[:, :D], ps_a[:])
                nc.scalar.copy(cs[:, D:], ps_b[:])
                gch, gof = g // CHW, (g % CHW) * P
                nc.sync.dma_start(combA_in[gch][gof:gof + P, :], cs[:])
                # local dst tables (bias folded here)
                ps_c = pbig.tile([P, D], F32, space='PSUM', tag='pbig')
                ps_d = pbig.tile([P, D], F32, space='PSUM', tag='pbig')
                for c in range(4):
                    nc.tensor.matmul(ps_c[:], lhsT=nft[:, c], rhs=wedst[c],
                                     start=(c == 0), stop=False)
                bias_mm(ps_c[:], 'b_e')
                for c in range(3):
                    nc.tensor.matmul(ps_d[:], lhsT=w2t[:, c], rhs=weldst[c],
                                     start=(c == 0), stop=False)
                bias_mm(ps_d[:], 'b_el')
                ds_ = wn.tile([P, 2 * D], B16, tag='bigcopy')
                nc.scalar.copy(ds_[:, :D], ps_c[:])
                nc.scalar.copy(ds_[:, D:], ps_d[:])
                nc.sync.dma_start(dstC_dram[g * P:(g + 1) * P, :], ds_[:])
                if (g + 1) % CHW == 0 or g == NW - 1:
                    ch = g // CHW
                    nc.gpsimd.collective_compute(
                        'AllGather', OP.bypass,
                        replica_groups=[list(range(NCORES))],
                        ins=[combA_in[ch][:]],
                        outs=[combA[ch * CRO:(ch + 1) * CRO, :]])

            if DEBUG:
                for blk in range(4):
                    stg = wn.tile([P, 2 * D], B16, tag='bigcopy')
                    nc.sync.dma_start(stg[:], combA[blk * P:(blk + 1) * P, :])
                    nc.sync.dma_start(dbg_comb[blk * P:(blk + 1) * P, :], stg[:])

            # node-MLP / readout weights into the phase-0 slots
            wnnf = wload(Wn_nf, D, D, 'wA')
            wnz = wload(Wn_z, D, D, 'wC')
            wr1ws = wload(Wr1_wsrc, DWP, D, 'wB')
            wr1wd = wload(Wr1_wdst, DWP, D, 'wD')

            # ============ PHASE 1+2: software-pipelined window loop.
            # Iteration w runs: exp+scale(w-1) -> agg MMs(w-1) ->
            # readout(w-2) -> norm+zzl(w-1) -> pass A(w) -> node MLP(w-1).
            # This keeps the PE queue dense: DVE/ACT/DMA work of one window
            # hides under the PE matmuls of its neighbours.
            st12 = {}          # per-window live tiles

            def w_passA(w):
                wsz = WSZ[w]
                t0 = sum(kw[:w])
                K = kw[w]
                lo = w * P
                s = st12[w] = {}
                nwin = wn.tile([P, 4, P], B16, tag='nft')
                s['nwin'] = nwin
                wwin = wn.tile([P, 3, P], B16, tag='w2t')
                s['wwin'] = wwin
                if wsz < P:
                    nc.vector.memset(nwin[:], 0.0)
                    nc.vector.memset(wwin[:], 0.0)
                nc.sync.dma_start(
                    nwin[:, :, :wsz],
                    nfT_loc[:, lo:lo + wsz].rearrange('(c p) n -> p c n', p=P))
                nc.sync.dma_start(
                    wwin[:, :, :wsz],
                    w2vT_loc[:, lo:lo + wsz].rearrange('(c p) n -> p c n', p=P))
                dstC = wn.tile([P, 2 * D], B16, tag='dstC')
                nc.sync.dma_start(dstC[:], dstC_dram[lo:lo + P, :])
                sftw = wb.tile([DS, K * P], B16, tag='sftw')
                nc.sync.dma_start(sftw[:], sfT_cols[:, t0 * P:(t0 + K) * P])
                dblc = wb.tile([P, K * P], B16, tag='dblc')
                nc.sync.dma_start(
                    dblc[:],
                    dstl_rows[:, t0 * P:(t0 + K) * P].to_broadcast((P, K * P)))
                wvl = wb.tile([P, K * WV], B16, tag='wvl')
                s['wvl'] = wvl
                nc.sync.dma_start(wvl[:], wv_cols[:, t0 * WV:(t0 + K) * WV])
                dstl = wn.tile([P, K], B16, tag='dstl')
                nc.sync.dma_start(dstl[:], dstl_cols[:, t0:t0 + K])
                # one-hot M [e,n] and MT [n,e] for ALL K tiles in 2 ops
                M_all = wb.tile([P, K * P], B16, tag='M_all')
                s['M_all'] = M_all
                nc.vector.tensor_tensor(
                    out=M_all[:].rearrange('p (k c) -> p k c', k=K),
                    in0=dstl[:, 0:K].to_broadcast((P, K, P)),
                    in1=colidxK[:, :K * P].rearrange('p (k c) -> p k c', k=K),
                    op=OP.is_equal)
                MT_all = wb.tile([P, K * P], B16, tag='MT_all')
                nc.vector.tensor_scalar(out=MT_all[:], in0=dblc[:],
                                        scalar1=pcol[:], scalar2=None,
                                        op0=OP.is_equal)
                efc = efp.tile([P, K, D], B16, tag='efc')
                s['efc'] = efc
                av = w1.tile([P, 2 * K], F32, tag='av')
                s['av'] = av
                for g0 in range(0, K, GRP):
                    G = min(GRP, K - g0)
                    gth = gp.tile([P, G, 2 * D], F8, tag=f'gath{G}')
                    nc.gpsimd.dma_gather(
                        out_ap=gth[:], in_ap=combA[:],
                        idxs_ap=gidx[:, g8[w] + g0 * 8:g8[w] + (g0 + G) * 8],
                        num_idxs=G * P, num_idxs_reg=G * P,
                        elem_size=2 * D)
                    if DEBUG and w == 0 and g0 == 0:
                        nc.sync.dma_start(
                            dbg_gth[:], gth[:].rearrange('p g e -> p (g e)'))
                    for tt_ in range(G):
                        t = g0 + tt_
                        pef_a = pbig.tile([P, D], F32, space='PSUM', tag='pbig')
                        pef_b = pbig.tile([P, D], F32, space='PSUM', tag='pbig')
                        nc.tensor.matmul(pef_a[:], lhsT=MT_all[:, t * P:(t + 1) * P],
                                         rhs=dstC[:, :D], start=True, stop=False)
                        nc.tensor.matmul(pef_a[:], lhsT=sftw[:, t * P:(t + 1) * P],
                                         rhs=wesf[0], start=False, stop=False)
                        nc.tensor.matmul(pef_a[:], lhsT=ident[:],
                                         rhs=gth[:, tt_, :D], start=False, stop=True)
                        nc.tensor.matmul(pef_b[:], lhsT=MT_all[:, t * P:(t + 1) * P],
                                         rhs=dstC[:, D:], start=True, stop=False)
                        nc.tensor.matmul(pef_b[:], lhsT=ident[:],
                                         rhs=gth[:, tt_, D:], start=False, stop=True)
                        # relu on ACT; signed logit accums on DVE
                        nc.scalar.activation(out=efc[:, t], in_=pef_a[:],
                                             func=AF.Relu)
                        scrB = ep.tile([P, D], B16, tag='scrB')
                        nc.scalar.activation(out=scrB[:], in_=pef_b[:],
                                             func=AF.Relu)
                        nc.vector.scalar_tensor_tensor(
                            out=scrB[:], in0=scrB[:], scalar=1.0, op0=OP.mult,
                            in1=Wal_b[:], op1=OP.mult,
                            accum_out=av[:, K + t:K + t + 1])
                        nc.vector.scalar_tensor_tensor(
                            out=scrB[:], in0=efc[:, t], scalar=1.0, op0=OP.mult,
                            in1=Wa_b[:], op1=OP.mult,
                            accum_out=av[:, t:t + 1])

            def w_evscale(w):
                s = st12[w]
                K = kw[w]
                av, efc, wvl = s['av'], s['efc'], s['wvl']
                # leaky relu + exp over all logits of the window
                lr1 = w1.tile([P, 2 * K], F32, tag='lr1')
                nc.vector.tensor_scalar(out=lr1[:], in0=av[:], scalar1=0.0,
                                        scalar2=0.2, op0=OP.min, op1=OP.mult)
                nc.vector.scalar_tensor_tensor(
                    out=av[:], in0=av[:], scalar=0.0, op0=OP.max,
                    in1=lr1[:], op1=OP.add)
                ev = w1.tile([P, 2 * K], F32, tag='ev')
                s['ev'] = ev
                nc.scalar.activation(out=ev[:], in_=av[:], func=AF.Exp)
                # scale efc / wvl in place (frees pass B to be pure PE)
                for t in range(K):
                    nc.vector.tensor_scalar(out=efc[:, t], in0=efc[:, t],
                                            scalar1=ev[:, t:t + 1],
                                            scalar2=None, op0=OP.mult)
                    nc.vector.tensor_scalar(out=wvl[:, t * WV:t * WV + DW + 1],
                                            in0=wvl[:, t * WV:t * WV + DW + 1],
                                            scalar1=ev[:, K + t:K + t + 1],
                                            scalar2=None, op0=OP.mult)
                    nc.vector.tensor_copy(wvl[:, t * WV + DW + 1:t * WV + DW + 2],
                                          ev[:, t:t + 1])

            def w_aggmm(w):
                s = st12[w]
                K = kw[w]
                efc, wvl, M_all = s['efc'], s['wvl'], s['M_all']
                agg = pagg.tile([P, 2 * D], F32, space='PSUM', tag='agg')
                s['agg'] = agg
                for t in range(K):
                    st_ = (t == 0)
                    sp = (t == K - 1)
                    nc.tensor.matmul(agg[:, :D], lhsT=M_all[:, t * P:(t + 1) * P],
                                     rhs=efc[:, t], start=st_, stop=sp)
                    nc.tensor.matmul(agg[:, D:D + WV],
                                     lhsT=M_all[:, t * P:(t + 1) * P],
                                     rhs=wvl[:, t * WV:(t + 1) * WV],
                                     start=st_, stop=sp)

            def w_norm(w):
                s = st12[w]
                agg = s['agg']
                rc = w1.tile([P, 2], F32, tag='rc')
                nc.vector.tensor_scalar_add(rc[:, 0:1],
                                            agg[:, D + DW + 1:D + DW + 2], 1e-9)
                nc.vector.tensor_scalar_add(rc[:, 1:2],
                                            agg[:, D + DW:D + DW + 1], 1e-9)
                nc.vector.reciprocal(rc[:], rc[:])
                zzl = w1.tile([P, D + DWP], B16, tag='zzl')
                nc.vector.tensor_scalar(out=zzl[:, :D], in0=agg[:, :D],
                                        scalar1=rc[:, 0:1], scalar2=None,
                                        op0=OP.mult)
                nc.vector.memset(zzl[:, D + DW:], 0.0)
                nc.vector.tensor_scalar(out=zzl[:, D:D + DW],
                                        in0=agg[:, D:D + DW],
                                        scalar1=rc[:, 1:2], scalar2=None,
                                        op0=OP.mult)
                zzlT = w1.tile([P, 7, P], B16, tag='zzlT')
                s['zzlT'] = zzlT
                nc.sync.dma_start_transpose(zzlT[:], zzl[:])

            def w_nodemlp(w):
                s = st12[w]
                zzlT, nwin, wwin = s['zzlT'], s['nwin'], s['wwin']
                zT = [zzlT[:, c] for c in range(4)]
                zlT = [zzlT[:, 4 + c] for c in range(3)]
                pnn_a = pbig.tile([P, D], F32, space='PSUM', tag='pbig')
                pnn_b = pbig.tile([P, D], F32, space='PSUM', tag='pbig')
                for c in range(4):
                    nc.tensor.matmul(pnn_a[:], lhsT=nwin[:, c], rhs=wnnf[c],
                                     start=(c == 0), stop=False)
                for c in range(4):
                    nc.tensor.matmul(pnn_a[:], lhsT=zT[c], rhs=wnz[c],
                                     start=False, stop=False)
                bias_mm(pnn_a[:], 'b_n')
                for c in range(3):
                    nc.tensor.matmul(pnn_b[:, :DW], lhsT=wwin[:, c],
                                     rhs=wnlw[c], start=(c == 0), stop=False)
                for c in range(3):
                    nc.tensor.matmul(pnn_b[:, :DW], lhsT=zlT[c],
                                     rhs=wnlz[c], start=False, stop=False)
                bias_mm(pnn_b[:, :DW], 'b_nl')
                nw_ = w1.tile([P, D + DWP], B16, tag='nw_')
                nc.scalar.activation(out=nw_[:, :D], in_=pnn_a[:], func=AF.Relu)
                nc.vector.memset(nw_[:, D + DW:], 0.0)
                nc.scalar.activation(out=nw_[:, D:D + DW], in_=pnn_b[:, :DW],
                                     func=AF.Relu)
                nwT = w1.tile([P, 7, P], B16, tag='nwT')
                s['nwT'] = nwT
                nc.sync.dma_start_transpose(nwT[:], nw_[:])

            def w_readout(w):
                s = st12[w]
                nwT = s['nwT']
                lo = w * P
                nnT = [nwT[:, c] for c in range(4)]
                wnT = [nwT[:, 4 + c] for c in range(3)]
                psr_a = pbig.tile([P, D], F32, space='PSUM', tag='pbig')
                psr_b = pbig.tile([P, D], F32, space='PSUM', tag='pbig')
                for c in range(4):
                    nc.tensor.matmul(psr_a[:], lhsT=nnT[c], rhs=wr1ns[c],
                                     start=(c == 0), stop=False)
                for c in range(3):
                    nc.tensor.matmul(psr_a[:], lhsT=wnT[c], rhs=wr1ws[c],
                                     start=False, stop=(c == 2))
                for c in range(4):
                    nc.tensor.matmul(psr_b[:], lhsT=nnT[c], rhs=wr1nd[c],
                                     start=(c == 0), stop=False)
                for c in range(3):
                    nc.tensor.matmul(psr_b[:], lhsT=wnT[c], rhs=wr1wd[c],
                                     start=False, stop=False)
                bias_mm(psr_b[:], 'b_r1')
                srt = wn.tile([P, 2 * D], B16, tag='bigcopy')
                nc.scalar.copy(srt[:, :D], psr_a[:])
                nc.scalar.copy(srt[:, D:], psr_b[:])
                ch, cof = w // CHW, (w % CHW) * P
                nc.sync.dma_start(srcR_in[ch][cof:cof + P, :], srt[:, :D])
                nc.sync.dma_start(dstR_dram[w * P:(w + 1) * P, :], srt[:, D:])
                if (w + 1) % CHW == 0 or w == NW - 1:
                    nc.gpsimd.collective_compute(
                        'AllGather', OP.bypass,
                        replica_groups=[list(range(NCORES))],
                        ins=[srcR_in[ch][:]],
                        outs=[srcR_full[ch * CRO:(ch + 1) * CRO, :]])
                del st12[w]

            for w in range(NW + 2):
                if 1 <= w <= NW:
                    w_aggmm(w - 1)
                if 2 <= w <= NW + 1:
                    w_readout(w - 2)
                if 1 <= w <= NW:
                    w_norm(w - 1)
                if w < NW:
                    w_passA(w)
                    w_evscale(w)
                if 1 <= w <= NW:
                    w_nodemlp(w - 1)

            if DEBUG:
                for blk in range(2):
                    stg = wn.tile([P, D], B16, tag='dstRw')
                    nc.sync.dma_start(stg[:], srcR_full[blk * P:(blk + 1) * P, :])
                    nc.sync.dma_start(dbg_srcR[blk * P:(blk + 1) * P, :], stg[:])
                stg = wn.tile([P, D], B16, tag='dstRw')
                nc.sync.dma_start(stg[:], dstR_dram[0:P, :])
                nc.sync.dma_start(dbg_dstR[:], stg[:])

            # ============ PHASE 3 per window (transposed frame)
            for w in range(NW):
                t0 = sum(kw[:w])
                K = kw[w]
                dstRw = wn.tile([P, D], B16, tag='dstRw')
                nc.sync.dma_start(dstRw[:], dstR_dram[w * P:(w + 1) * P, :])
                sftw = wb.tile([DS, K * P], B16, tag='sftw')
                nc.sync.dma_start(sftw[:], sfT_cols[:, t0 * P:(t0 + K) * P])
                dblc = wb.tile([P, K * P], B16, tag='dblc')
                nc.sync.dma_start(
                    dblc[:],
                    dstl_rows[:, t0 * P:(t0 + K) * P].to_broadcast((P, K * P)))
                MT_all = wb.tile([P, K * P], B16, tag='MT_all')
                nc.vector.tensor_scalar(out=MT_all[:], in0=dblc[:],
                                        scalar1=pcol[:], scalar2=None,
                                        op0=OP.is_equal)
                outT = wo.tile([NCLS, K * P], B16, tag='outT')
                for q0 in range(0, K, GRP3):
                    Q = min(GRP3, K - q0)
                    ghT = p3.tile([P, 4, Q * P], B16, tag=f'ghT{Q}')
                    nc.gpsimd.dma_gather(
                        out_ap=ghT[:], in_ap=srcR_full[:],
                        idxs_ap=gidx[:, g8[w] + q0 * 8:g8[w] + (q0 + Q) * 8],
                        num_idxs=Q * P, num_idxs_reg=Q * P,
                        elem_size=D, transpose=True)
                    for g0 in range(q0, q0 + Q, GRP):
                        G = min(GRP, q0 + Q - g0)
                        ec = G * P
                        e0 = g0 * P
                        eg = (g0 - q0) * P
                        hT = p3.tile([P, 4, GRP * P], B16, tag='hT')
                        for c in range(4):
                            pr = pbig.tile([P, GRP * P], F32, space='PSUM',
                                           tag='pbig')
                            nc.tensor.matmul(pr[:, :ec],
                                             lhsT=dstRw[:, c * P:(c + 1) * P],
                                             rhs=MT_all[:, e0:e0 + ec],
                                             start=True, stop=False)
                            nc.tensor.matmul(pr[:, :ec],
                                             lhsT=wr1sf[0][:, c * P:(c + 1) * P],
                                             rhs=sftw[:, e0:e0 + ec],
                                             start=False, stop=False)
                            nc.tensor.matmul(pr[:, :ec], lhsT=ident[:],
                                             rhs=ghT[:, c, eg:eg + ec],
                                             start=False, stop=True)
                            nc.scalar.activation(out=hT[:, c, :ec],
                                                 in_=pr[:, :ec], func=AF.Relu)
                        if DEBUG and w == 0 and g0 == 0:
                            nc.sync.dma_start(
                                dbg_hT[:], hT[:].rearrange('p c e -> p (c e)'))
                        po = psml.tile([NCLS, GRP * P], F32, space='PSUM',
                                       tag='po2')
                        for c in range(4):
                            nc.tensor.matmul(po[:, :ec], lhsT=wr2[c],
                                             rhs=hT[:, c, :ec],
                                             start=(c == 0), stop=(c == 3))
                        nc.vector.tensor_scalar(
                            out=outT[:, e0:e0 + ec],
                            in0=po[:, :ec], scalar1=br2c[:], scalar2=None,
                            op0=OP.add)
                nc.sync.dma_start(out_colsT[:, t0 * P:(t0 + K) * P], outT[:])
    return nc


# ----------------------------------------------------------------- weights
def _weights(W_e, b_e, W_el, b_el, W_a, b_a, W_al, b_al, W_n, b_n, W_nl, b_nl,
             W_r1, b_r1, W_r2, b_r2):
    def padrows(m, rows):
        out = np.zeros((rows, m.shape[1]), np.float32)
        out[:m.shape[0]] = m
        return out

    return {
        'We_src': np.ascontiguousarray(np.asarray(W_e[0:D], np.float32)),
        'We_sf': np.ascontiguousarray(np.asarray(W_e[D:D + DS], np.float32)),
        'We_dst': np.ascontiguousarray(np.asarray(W_e[D + DS:], np.float32)),
        'Wel_src': padrows(W_el[0:DW], DWP),
        'Wel_dst': padrows(W_el[DW:], DWP),
        'Wn_nf': np.ascontiguousarray(W_n[0:D]),
        'Wn_z': np.ascontiguousarray(np.asarray(W_n[D:], np.float32)),
        'Wnl_w': padrows(W_nl[0:DW], DWP),
        'Wnl_z': padrows(W_nl[DW:], DWP),
        'Wr1_nsrc': np.ascontiguousarray(W_r1[0:D]),
        'Wr1_wsrc': padrows(W_r1[D:D + DW], DWP),
        'Wr1_sf': np.ascontiguousarray(W_r1[D + DW:D + DW + DS]),
        'Wr1_wdst': padrows(W_r1[D + DW + DS:D + 2 * DW + DS], DWP),
        'Wr1_ndst': np.ascontiguousarray(W_r1[D + 2 * DW + DS:]),
        'W_r2': np.ascontiguousarray(W_r2),
        'b_e': np.asarray(b_e, np.float32).reshape(1, D),
        'b_el': b_el.reshape(1, D),
        'b_n': b_n.reshape(1, D), 'b_nl': b_nl.reshape(1, DW),
        'b_r1': b_r1.reshape(1, D),
        'b_r2col': np.asarray(b_r2, np.float32).reshape(NCLS, 1),
        'Wal_row': W_al.reshape(1, D),
        'Wa_row': W_a.reshape(1, D),
    }, D


def _make_in_maps(np_inputs, per_core):
    import ml_dtypes
    bf16 = ml_dtypes.bfloat16
    n_f = np.asarray(np_inputs['n_f'], np.float32)
    word2vec = np.asarray(np_inputs['word2vec'], np.float32)
    n_fT = np.ascontiguousarray(n_f.T).astype(bf16)
    w2vT = np.zeros((DWP, N), np.float32)
    w2vT[:DW] = word2vec.T
    w2vT = w2vT.astype(bf16)
    wts, _pa = _weights(*[np.asarray(np_inputs[k], np.float32) for k in
                          ('W_e', 'b_e', 'W_el', 'b_el', 'W_a', 'b_a', 'W_al',
                           'b_al', 'W_n', 'b_n', 'W_nl', 'b_nl', 'W_r1',
                           'b_r1', 'W_r2', 'b_r2')])
    wts = {k: (v if k == 'b_r2col' else v.astype(bf16))
           for k, v in wts.items()}
    in_maps = []
    for r in range(NCORES):
        m = {'nfT_loc': np.ascontiguousarray(n_fT[:, r * NPC:(r + 1) * NPC]),
             'w2vT_loc': np.ascontiguousarray(w2vT[:, r * NPC:(r + 1) * NPC])}
        m.update(wts)
        pc = per_core[r]
        m.update({'gidx': pc['gidx'],
                  'dstl_cols': pc['dstl_cols'].astype(bf16),
                  'dstl_rows': pc['dstl_rows'].astype(bf16),
                  'sfT_cols': pc['sfT_cols'].astype(bf16),
                  'wv_cols': pc['wv_cols'].astype(bf16)})
        in_maps.append(m)
    return in_maps


def kernel(n_f, word2vec, s_f, src, dst, W_e, b_e, W_el, b_el, W_a, b_a,
           W_al, b_al, W_n, b_n, W_nl, b_nl, W_r1, b_r1, W_r2, b_r2):
    np_inputs = dict(n_f=n_f, word2vec=word2vec, s_f=s_f, src=src, dst=dst,
                     W_e=W_e, b_e=b_e, W_el=W_el, b_el=b_el, W_a=W_a, b_a=b_a,
                     W_al=W_al, b_al=b_al, W_n=W_n, b_n=b_n, W_nl=W_nl,
                     b_nl=b_nl, W_r1=W_r1, b_r1=b_r1, W_r2=W_r2, b_r2=b_r2)
    src = np.asarray(src, np.int32)
    dst = np.asarray(dst, np.int32)
    s_f = np.asarray(s_f, np.float32)

    per_core, kw, T = _prep(src, dst, s_f,
                            np.asarray(word2vec, np.float32))
    dp = _lang_split(np_inputs['W_a'])[2]
    nc = _build(T, kw, dp=dp)
    nc.compile()
    in_maps = _make_in_maps(np_inputs, per_core)

    res = run_bass_kernel_spmd(nc, in_maps, core_ids=list(range(NCORES)))

    out = np.zeros((E, NCLS), np.float32)
    for r in range(NCORES):
        oc = np.asarray(res.results[r]['out_colsT'], np.float32)
        vals = np.ascontiguousarray(oc.T)                    # [T*P, NCLS]
        eid = per_core[r]['eid'].reshape(-1)
        mask = eid >= 0
        out[eid[mask]] = vals[mask]
    return out
